# revision 14
# baseline (speedup 1.0000x reference)
"""LocallyConnected2D (B=16, H=W=64, C=32, 3x3 valid, F=64) on 8 trn2 cores.

out[b, oh, ow, f] = sum_{kh,kw,c} x[b, oh+kh, ow+kw, c] * kernel[p, (kh,kw,c), f] + bias[p, f]
with p = oh*62+ow.  P=3844 sharded by oh-rows across 8 cores (8 rows/core,
core 7 padded).

Per core: weights stream from HBM in fp16 as one flat [97 x 95232] tensor
(partition row 96 = bias folded into the kh=0 chunk), pulled in per-granule
DMAs on the SP queue only, so nothing ever stalls the weight stream.
Patches are pre-transposed on the host into [97 x 992] fp16 tiles (row 96 =
ones); x and output DMAs ride the Activation queue.  Each position runs 3
stationary matmuls ([97,64] weights stationary, 16 batch columns moving)
accumulating into PSUM; each granule's PSUM block is cast/copied to fp16
SBUF and written out f-major (host unscrambles).  The final granules are
small and their stores deferred so the post-stream dependency chain is
minimal.
"""

import sys

for _p in ("/opt/trn_rl_repo",):
    if _p not in sys.path:
        sys.path.insert(0, _p)

import numpy as np
from contextlib import ExitStack

import concourse.bass as bass
import concourse.bacc as bacc
import concourse.mybir as mybir
import concourse.tile as tile
from concourse.bass_utils import run_bass_kernel_spmd

F32 = mybir.dt.float32
F16 = mybir.dt.float16

B, H, W, C = 16, 64, 64, 32
KH, KW = 3, 3
OH, OW = 62, 62
F = 64
NCORES = 8
RPC = 8            # oh rows per core (core 7: 2 rows are padding)
NXR = RPC + 2      # x rows staged per core
PPC = RPC * OW     # 496 positions per core (padded for core 7)
KP = KW * C + 1    # 97 partitions: 96 contraction rows + bias/ones row
CPP = KH * F       # 192 weight columns per position

# (oh, ow0, npos) granules; the last one is tiny to shrink the tail chain.
GRANULES = []
for _oh in range(RPC - 1):
    GRANULES.append((_oh, 0, 31))
    GRANULES.append((_oh, 31, 31))
GRANULES += [(RPC - 1, 0, 31), (RPC - 1, 31, 27), (RPC - 1, 58, 4)]
# copies+stores of granules [-N_DEFER-1 .. -2] issue after the last granule's
# weight DMA so the weight stream ends as early as possible.
N_DEFER = 4

_cached = {}


def _build_program():
    if "nc" in _cached:
        return _cached["nc"]

    nc = bacc.Bacc(None)
    # xt[r, kw*32+c, ow*16+b] = x[b, r0+r, ow+kw, c]; row 96 = 1.0
    xt = nc.declare_dram_parameter("xt", [NXR, KP, OW * B], F16, isOutput=False)
    # ks[kw*32+c, p'*192 + ch*64 + f] = kernel[p0+p', ch*96+kw*32+c, f];
    # row 96: bias at ch==0, zero at ch 1..2
    ks = nc.declare_dram_parameter("ks", [KP, PPC * CPP], F16, isOutput=False)
    # out[f, p'*16+b]
    out = nc.declare_dram_parameter("out", [F, PPC * B], F16, isOutput=True)

    with ExitStack() as ctx:
        tc = ctx.enter_context(tile.TileContext(nc))
        tpool = ctx.enter_context(tc.tile_pool(name="tpool", bufs=NXR))
        ktpool = ctx.enter_context(tc.tile_pool(name="ktpool", bufs=4))
        pspool = ctx.enter_context(tc.tile_pool(name="pspool", bufs=8, space="PSUM"))
        stpool = ctx.enter_context(tc.tile_pool(name="stpool", bufs=N_DEFER + 4))

        T = []
        for r in range(NXR):
            t_tile = tpool.tile([KP, OW * B], F16)
            T.append(t_tile)
        for r in range(KH):
            nc.scalar.dma_start(T[r][:, :], xt[r])

        def emit_matmuls(ps, kt, oh, ow0, npos):
            for owl in range(npos):
                ow = ow0 + owl
                for ch in range(KH):
                    nc.tensor.matmul(
                        ps[0:F, owl * B : (owl + 1) * B],
                        kt[0:KP, (owl * KH + ch) * F : (owl * KH + ch + 1) * F],
                        T[oh + ch][0:KP, ow * B : (ow + 1) * B],
                        start=(ch == 0),
                        stop=(ch == KH - 1),
                    )

        def emit_store(ps, oh, ow0, npos, last=False):
            pbase = oh * OW + ow0
            st = stpool.tile([F, 31 * B], F16)
            nc.vector.tensor_copy(st[:, : npos * B], ps[:, : npos * B])
            dst = out[:, pbase * B : (pbase + npos) * B]
            # the final store goes on the (by then idle) SP queue
            eng = nc.sync if last else nc.scalar
            eng.dma_start(dst, st[:, : npos * B])

        deferred = []
        seen_rows = KH
        for gi, (oh, ow0, npos) in enumerate(GRANULES):
            pbase = oh * OW + ow0
            kt = ktpool.tile([KP, 31 * CPP], F16)
            nc.sync.dma_start(
                kt[:, : npos * CPP],
                ks[:, pbase * CPP : (pbase + npos) * CPP],
            )
            if oh + KH >= seen_rows and seen_rows < NXR:
                nc.scalar.dma_start(T[seen_rows][:, :], xt[seen_rows])
                seen_rows += 1
            ps = pspool.tile([F, 31 * B], F32)
            emit_matmuls(ps, kt, oh, ow0, npos)
            if len(GRANULES) - 1 - N_DEFER <= gi < len(GRANULES) - 1:
                deferred.append((ps, oh, ow0, npos))
            else:
                emit_store(ps, oh, ow0, npos, last=(gi == len(GRANULES) - 1))
        for ps, oh, ow0, npos in deferred:
            emit_store(ps, oh, ow0, npos)

    nc.finalize()
    _cached["nc"] = nc
    return nc


def _shard_inputs(x, kernel, bias):
    x = np.asarray(x, dtype=np.float32)
    kernel = np.asarray(kernel, dtype=np.float32)
    bias = np.asarray(bias, dtype=np.float32)
    kernel16 = kernel.astype(np.float16)   # (P, 288, 64)
    bias16 = bias.astype(np.float16)       # (P, 64)
    x16 = x.astype(np.float16)             # (B, H, W, C)

    in_maps = []
    for c in range(NCORES):
        r0 = RPC * c
        nrows = min(NXR, H - r0)
        xs_c = np.zeros((NXR, B, W, C), dtype=np.float16)
        xs_c[:nrows] = np.moveaxis(x16[:, r0 : r0 + nrows], 1, 0)

        xt_c = np.empty((NXR, KP, OW * B), dtype=np.float16)
        xt_c[:, KP - 1, :] = np.float16(1.0)
        for kw in range(KW):
            # (NXR, B, OW, C) -> (NXR, C, OW, B)
            blk = xs_c[:, :, kw : kw + OW, :].transpose(0, 3, 2, 1)
            xt_c[:, kw * C : (kw + 1) * C, :] = blk.reshape(NXR, C, OW * B)

        p0 = PPC * c
        pe = min(p0 + PPC, OH * OW)
        npos = pe - p0
        kblk = np.zeros((PPC, KH, KW * C, F), dtype=np.float16)
        kblk[:npos] = kernel16[p0:pe].reshape(npos, KH, KW * C, F)
        # ks[kwc, p*192 + ch*64 + f]
        ks_c = np.zeros((KP, PPC * CPP), dtype=np.float16)
        ks_c[: KW * C] = kblk.transpose(2, 0, 1, 3).reshape(KW * C, PPC * CPP)
        brow = np.zeros((PPC, KH, F), dtype=np.float16)
        brow[:npos, 0, :] = bias16[p0:pe]
        ks_c[KP - 1] = brow.reshape(PPC * CPP)

        in_maps.append({"xt": xt_c, "ks": ks_c})
    return in_maps


def _run(x, kernel, bias, trace=False):
    nc = _build_program()
    in_maps = _shard_inputs(x, kernel, bias)
    res = run_bass_kernel_spmd(nc, in_maps, core_ids=list(range(NCORES)), trace=trace)
    out_full = np.empty((B, OH, OW, F), dtype=np.float32)
    for c in range(NCORES):
        rows = min(RPC, OH - RPC * c)
        o = np.asarray(res.results[c]["out"], dtype=np.float32)  # (64, 7936)
        # (f, p', b) -> (b, oh, ow, f)
        o = o.reshape(F, RPC, OW, B).transpose(3, 1, 2, 0)
        out_full[:, RPC * c : RPC * c + rows] = o[:, :rows]
    return out_full, res


def kernel(x, kernel, bias):
    out, _ = _run(x, kernel, bias, trace=False)
    return out


# revision 15
# speedup vs baseline: 1.0347x; 1.0347x over previous
"""LocallyConnected2D (B=16, H=W=64, C=32, 3x3 valid, F=64) on 8 trn2 cores.

out[b, oh, ow, f] = sum_{kh,kw,c} x[b, oh+kh, ow+kw, c] * kernel[p, (kh,kw,c), f] + bias[p, f]
with p = oh*62+ow.  P=3844 sharded by oh-rows across 8 cores (8 rows/core,
core 7 padded).

Per core: weights stream from HBM in fp16 as one flat [97 x 95232] tensor
(partition row 96 = bias folded into the kh=0 chunk), pulled in per-granule
DMAs on the SP queue only, so nothing ever stalls the weight stream.
Patches are pre-transposed on the host into [97 x 992] fp16 tiles (row 96 =
ones); x and output DMAs ride the Activation queue.  Each position runs 3
stationary matmuls ([97,64] weights stationary, 16 batch columns moving)
accumulating into PSUM; each granule's PSUM block is cast/copied to fp16
SBUF and written out f-major (host unscrambles).  The final granules are
small and their stores deferred so the post-stream dependency chain is
minimal.
"""

import sys

for _p in ("/opt/trn_rl_repo",):
    if _p not in sys.path:
        sys.path.insert(0, _p)

import numpy as np
from contextlib import ExitStack

import concourse.bass as bass
import concourse.bacc as bacc
import concourse.mybir as mybir
import concourse.tile as tile
from concourse.bass_utils import run_bass_kernel_spmd

F32 = mybir.dt.float32
F16 = mybir.dt.float16

B, H, W, C = 16, 64, 64, 32
KH, KW = 3, 3
OH, OW = 62, 62
F = 64
NCORES = 8
RPC = 8            # oh rows per core (core 7: 2 rows are padding)
NXR = RPC + 2      # x rows staged per core
PPC = RPC * OW     # 496 positions per core (padded for core 7)
KP = KW * C + 1    # 97 partitions: 96 contraction rows + bias/ones row
CPP = KH * F       # 192 weight columns per position

# (oh, ow0, npos) granules; the last one is tiny to shrink the tail chain.
GRANULES = []
for _oh in range(RPC - 1):
    GRANULES.append((_oh, 0, 31))
    GRANULES.append((_oh, 31, 31))
GRANULES += [(RPC - 1, 0, 31), (RPC - 1, 31, 27), (RPC - 1, 58, 4)]
# copies+stores of granules [-N_DEFER-1 .. -2] issue after the last granule's
# weight DMA so the weight stream ends as early as possible.
N_DEFER = 4

_cached = {}


def _build_program():
    if "nc" in _cached:
        return _cached["nc"]

    nc = bacc.Bacc(None)
    # xt[r, kw*32+c, ow*16+b] = x[b, r0+r, ow+kw, c]; row 96 = 1.0
    xt = nc.declare_dram_parameter("xt", [NXR, KP, OW * B], F16, isOutput=False)
    # ks[kw*32+c, p'*192 + ch*64 + f] = kernel[p0+p', ch*96+kw*32+c, f];
    # row 96: bias at ch==0, zero at ch 1..2
    ks = nc.declare_dram_parameter("ks", [KP, PPC * CPP], F16, isOutput=False)
    # out[f, p'*16+b]
    out = nc.declare_dram_parameter("out", [F, PPC * B], F16, isOutput=True)

    with ExitStack() as ctx:
        tc = ctx.enter_context(tile.TileContext(nc))
        tpool = ctx.enter_context(tc.tile_pool(name="tpool", bufs=NXR))
        ktpool = ctx.enter_context(tc.tile_pool(name="ktpool", bufs=4))
        pspool = ctx.enter_context(tc.tile_pool(name="pspool", bufs=8, space="PSUM"))
        stpool = ctx.enter_context(tc.tile_pool(name="stpool", bufs=N_DEFER + 4))

        T = []
        for r in range(NXR):
            t_tile = tpool.tile([KP, OW * B], F16)
            T.append(t_tile)
        for r in range(KH):
            nc.scalar.dma_start(T[r][:, :], xt[r])

        def emit_matmuls(ps, kt, oh, ow0, npos):
            for owl in range(npos):
                ow = ow0 + owl
                for ch in range(KH):
                    nc.tensor.matmul(
                        ps[0:F, owl * B : (owl + 1) * B],
                        kt[0:KP, (owl * KH + ch) * F : (owl * KH + ch + 1) * F],
                        T[oh + ch][0:KP, ow * B : (ow + 1) * B],
                        start=(ch == 0),
                        stop=(ch == KH - 1),
                    )

        def emit_store(ps, oh, ow0, npos, last=False):
            pbase = oh * OW + ow0
            st = stpool.tile([F, 31 * B], F16)
            nc.vector.tensor_copy(st[:, : npos * B], ps[:, : npos * B])
            dst = out[:, pbase * B : (pbase + npos) * B]
            # the final store goes on the (by then idle) SP queue
            eng = nc.sync if last else nc.scalar
            eng.dma_start(dst, st[:, : npos * B])

        deferred = []
        seen_rows = KH
        for gi, (oh, ow0, npos) in enumerate(GRANULES):
            pbase = oh * OW + ow0
            kt = ktpool.tile([KP, 31 * CPP], F16)
            nc.sync.dma_start(
                kt[:, : npos * CPP],
                ks[:, pbase * CPP : (pbase + npos) * CPP],
            )
            if oh + KH >= seen_rows and seen_rows < NXR:
                nc.scalar.dma_start(T[seen_rows][:, :], xt[seen_rows])
                seen_rows += 1
            ps = pspool.tile([F, 31 * B], F32)
            emit_matmuls(ps, kt, oh, ow0, npos)
            emit_store(ps, oh, ow0, npos, last=(gi == len(GRANULES) - 1))

    nc.finalize()
    _cached["nc"] = nc
    return nc


def _shard_inputs(x, kernel, bias):
    x = np.asarray(x, dtype=np.float32)
    kernel = np.asarray(kernel, dtype=np.float32)
    bias = np.asarray(bias, dtype=np.float32)
    kernel16 = kernel.astype(np.float16)   # (P, 288, 64)
    bias16 = bias.astype(np.float16)       # (P, 64)
    x16 = x.astype(np.float16)             # (B, H, W, C)

    in_maps = []
    for c in range(NCORES):
        r0 = RPC * c
        nrows = min(NXR, H - r0)
        xs_c = np.zeros((NXR, B, W, C), dtype=np.float16)
        xs_c[:nrows] = np.moveaxis(x16[:, r0 : r0 + nrows], 1, 0)

        xt_c = np.empty((NXR, KP, OW * B), dtype=np.float16)
        xt_c[:, KP - 1, :] = np.float16(1.0)
        for kw in range(KW):
            # (NXR, B, OW, C) -> (NXR, C, OW, B)
            blk = xs_c[:, :, kw : kw + OW, :].transpose(0, 3, 2, 1)
            xt_c[:, kw * C : (kw + 1) * C, :] = blk.reshape(NXR, C, OW * B)

        p0 = PPC * c
        pe = min(p0 + PPC, OH * OW)
        npos = pe - p0
        kblk = np.zeros((PPC, KH, KW * C, F), dtype=np.float16)
        kblk[:npos] = kernel16[p0:pe].reshape(npos, KH, KW * C, F)
        # ks[kwc, p*192 + ch*64 + f]
        ks_c = np.zeros((KP, PPC * CPP), dtype=np.float16)
        ks_c[: KW * C] = kblk.transpose(2, 0, 1, 3).reshape(KW * C, PPC * CPP)
        brow = np.zeros((PPC, KH, F), dtype=np.float16)
        brow[:npos, 0, :] = bias16[p0:pe]
        ks_c[KP - 1] = brow.reshape(PPC * CPP)

        in_maps.append({"xt": xt_c, "ks": ks_c})
    return in_maps


def _run(x, kernel, bias, trace=False):
    nc = _build_program()
    in_maps = _shard_inputs(x, kernel, bias)
    res = run_bass_kernel_spmd(nc, in_maps, core_ids=list(range(NCORES)), trace=trace)
    out_full = np.empty((B, OH, OW, F), dtype=np.float32)
    for c in range(NCORES):
        rows = min(RPC, OH - RPC * c)
        o = np.asarray(res.results[c]["out"], dtype=np.float32)  # (64, 7936)
        # (f, p', b) -> (b, oh, ow, f)
        o = o.reshape(F, RPC, OW, B).transpose(3, 1, 2, 0)
        out_full[:, RPC * c : RPC * c + rows] = o[:, :rows]
    return out_full, res


def kernel(x, kernel, bias):
    out, _ = _run(x, kernel, bias, trace=False)
    return out


# revision 16
# speedup vs baseline: 1.0613x; 1.0257x over previous
"""LocallyConnected2D (B=16, H=W=64, C=32, 3x3 valid, F=64) on 8 trn2 cores.

out[b, oh, ow, f] = sum_{kh,kw,c} x[b, oh+kh, ow+kw, c] * kernel[p, (kh,kw,c), f] + bias[p, f]
with p = oh*62+ow, P = 3844.

P is sharded evenly: NPOS=481 consecutive positions per core (cores 4-7 pad
one).  The SPMD program is phase-independent via *virtual rows*: core-local
position j has virtual row v=j//62, column u=j%62, and because
oh(p-62)+1 == oh(p), one patch tile T[w] per virtual row serves as chunk ch
for v=w-ch regardless of the core's row phase — the phase lives entirely in
the host-side layout of T.

Per core: weights stream from HBM in fp16 as one flat [97 x NPOS*192] tensor
(partition row 96 = bias folded into the kh=0 chunk), pulled in per-granule
DMAs on the SP queue only so nothing stalls the weight stream; x and output
DMAs ride the Activation queue.  Each position runs 3 stationary matmuls
([97,64] weights stationary, 16 batch columns moving) accumulating into
PSUM; each granule is cast/copied to fp16 SBUF and stored f-major (host
unscrambles).  The final granules are small and their stores deferred to
minimize the post-stream dependency chain.
"""

import sys

for _p in ("/opt/trn_rl_repo",):
    if _p not in sys.path:
        sys.path.insert(0, _p)

import numpy as np
from contextlib import ExitStack

import concourse.bass as bass
import concourse.bacc as bacc
import concourse.mybir as mybir
import concourse.tile as tile
from concourse.bass_utils import run_bass_kernel_spmd

F32 = mybir.dt.float32
F16 = mybir.dt.float16

B, H, W, C = 16, 64, 64, 32
KH, KW = 3, 3
OH, OW = 62, 62
P_ALL = OH * OW    # 3844
F = 64
NCORES = 8
NPOS = 481         # positions per core (cores 4-7: 480 real + 1 pad)
NVR = 8            # virtual rows (7 full + 47-position partial)
NXR = NVR + 2      # patch tiles staged per core
KP = KW * C + 1    # 97 partitions: 96 contraction rows + bias/ones row
CPP = KH * F       # 192 weight columns per position

# (v, u0, npos) granules; the last ones are small to shrink the tail chain.
GRANULES = []
for _v in range(NVR - 1):
    GRANULES.append((_v, 0, 31))
    GRANULES.append((_v, 31, 31))
GRANULES += [(NVR - 1, 0, 31), (NVR - 1, 31, 12), (NVR - 1, 43, 4)]
assert sum(g[2] for g in GRANULES) == NPOS
N_DEFER = 4        # store DMAs of the last N_DEFER granules issue post-stream


def _p0(c):
    return 481 * c if c < 4 else 1924 + 480 * (c - 4)


_cached = {}


def _build_program():
    if "nc" in _cached:
        return _cached["nc"]

    nc = bacc.Bacc(None)
    # xt[w, kw*32+c, u*16+b] = x[b, w + (p0+u)//62, (p0+u)%62 + kw, c]; row 96 = 1.0
    xt = nc.declare_dram_parameter("xt", [NXR, KP, OW * B], F16, isOutput=False)
    # ks[kw*32+c, j*192 + ch*64 + f] = kernel[p0+j, ch*96+kw*32+c, f];
    # row 96: bias at ch==0, zero at ch 1..2
    ks = nc.declare_dram_parameter("ks", [KP, NPOS * CPP], F16, isOutput=False)
    # out[f, j*16+b]
    out = nc.declare_dram_parameter("out", [F, NPOS * B], F16, isOutput=True)

    with ExitStack() as ctx:
        tc = ctx.enter_context(tile.TileContext(nc))
        tpool = ctx.enter_context(tc.tile_pool(name="tpool", bufs=NXR))
        ktpool = ctx.enter_context(tc.tile_pool(name="ktpool", bufs=4))
        pspool = ctx.enter_context(tc.tile_pool(name="pspool", bufs=4, space="PSUM"))
        stpool = ctx.enter_context(tc.tile_pool(name="stpool", bufs=N_DEFER + 4))

        T = []
        for r in range(NXR):
            t_tile = tpool.tile([KP, OW * B], F16)
            T.append(t_tile)
        for r in range(KH):
            nc.scalar.dma_start(T[r][:, :], xt[r])

        deferred = []
        seen_rows = KH
        for gi, (v, u0, npos) in enumerate(GRANULES):
            j0 = v * OW + u0
            kt = ktpool.tile([KP, 31 * CPP], F16)
            nc.sync.dma_start(
                kt[:, : npos * CPP],
                ks[:, j0 * CPP : (j0 + npos) * CPP],
            )
            if v + KH >= seen_rows and seen_rows < NXR:
                nc.scalar.dma_start(T[seen_rows][:, :], xt[seen_rows])
                seen_rows += 1
            ps = pspool.tile([F, 31 * B], F32)
            for owl in range(npos):
                u = u0 + owl
                for ch in range(KH):
                    nc.tensor.matmul(
                        ps[0:F, owl * B : (owl + 1) * B],
                        kt[0:KP, (owl * KH + ch) * F : (owl * KH + ch + 1) * F],
                        T[v + ch][0:KP, u * B : (u + 1) * B],
                        start=(ch == 0),
                        stop=(ch == KH - 1),
                    )
            st = stpool.tile([F, 31 * B], F16)
            nc.vector.tensor_copy(st[:, : npos * B], ps[:, : npos * B])
            dst = out[:, j0 * B : (j0 + npos) * B]
            if gi >= len(GRANULES) - N_DEFER:
                deferred.append((dst, st, npos))
            else:
                nc.scalar.dma_start(dst, st[:, : npos * B])
        for di, (dst, st, npos) in enumerate(deferred):
            # the very last store rides the (by then idle) SP queue
            eng = nc.sync if di == len(deferred) - 1 else nc.scalar
            eng.dma_start(dst, st[:, : npos * B])

    nc.finalize()
    _cached["nc"] = nc
    return nc


def _shard_inputs(x, kernel, bias):
    x = np.asarray(x, dtype=np.float32)
    kernel = np.asarray(kernel, dtype=np.float32)
    bias = np.asarray(bias, dtype=np.float32)
    kernel16 = kernel.astype(np.float16)   # (P, 288, 64)
    bias16 = bias.astype(np.float16)       # (P, 64)
    x16 = x.astype(np.float16)             # (B, H, W, C)

    xpad = np.zeros((B, H + NXR, W, C), dtype=np.float16)
    xpad[:, :H] = x16

    in_maps = []
    for c in range(NCORES):
        p0 = _p0(c)
        u_arr = np.arange(OW)
        rbase = (p0 + u_arr) // OW   # (62,)
        ucol = (p0 + u_arr) % OW     # (62,)

        xt_c = np.empty((NXR, KP, OW * B), dtype=np.float16)
        xt_c[:, KP - 1, :] = np.float16(1.0)
        for w in range(NXR):
            rows = w + rbase         # (62,)
            for kw in range(KW):
                vals = xpad[:, rows, ucol + kw, :]        # (B, 62, C)
                xt_c[w, kw * C : (kw + 1) * C, :] = vals.transpose(2, 1, 0).reshape(
                    C, OW * B
                )

        nreal = min(NPOS, P_ALL - p0)
        kblk = np.zeros((NPOS, KH, KW * C, F), dtype=np.float16)
        kblk[:nreal] = kernel16[p0 : p0 + nreal].reshape(nreal, KH, KW * C, F)
        ks_c = np.zeros((KP, NPOS * CPP), dtype=np.float16)
        ks_c[: KW * C] = kblk.transpose(2, 0, 1, 3).reshape(KW * C, NPOS * CPP)
        brow = np.zeros((NPOS, KH, F), dtype=np.float16)
        brow[:nreal, 0, :] = bias16[p0 : p0 + nreal]
        ks_c[KP - 1] = brow.reshape(NPOS * CPP)

        in_maps.append({"xt": xt_c, "ks": ks_c})
    return in_maps


def _run(x, kernel, bias, trace=False):
    nc = _build_program()
    in_maps = _shard_inputs(x, kernel, bias)
    res = run_bass_kernel_spmd(nc, in_maps, core_ids=list(range(NCORES)), trace=trace)
    out_flat = np.empty((P_ALL, B, F), dtype=np.float32)
    for c in range(NCORES):
        p0 = _p0(c)
        nreal = min(NPOS, P_ALL - p0)
        o = np.asarray(res.results[c]["out"], dtype=np.float32)  # (64, NPOS*16)
        o = o.reshape(F, NPOS, B).transpose(1, 2, 0)             # (j, b, f)
        out_flat[p0 : p0 + nreal] = o[:nreal]
    out_full = out_flat.reshape(OH, OW, B, F).transpose(2, 0, 1, 3)
    return np.ascontiguousarray(out_full), res


def kernel(x, kernel, bias):
    out, _ = _run(x, kernel, bias, trace=False)
    return out


# revision 18
# speedup vs baseline: 1.0635x; 1.0020x over previous
"""LocallyConnected2D (B=16, H=W=64, C=32, 3x3 valid, F=64) on 8 trn2 cores.

out[b, oh, ow, f] = sum_{kh,kw,c} x[b, oh+kh, ow+kw, c] * kernel[p, (kh,kw,c), f] + bias[p, f]
with p = oh*62+ow, P = 3844.

P is sharded evenly: NPOS=481 consecutive positions per core (cores 4-7 pad
one).  The SPMD program is phase-independent via *virtual rows*: core-local
position j has virtual row v=j//62, column u=j%62, and because
oh(p-62)+1 == oh(p), one patch tile T[w] per virtual row serves as chunk ch
for v=w-ch regardless of the core's row phase — the phase lives entirely in
the host-side layout of T.

Per core: weights stream from HBM in fp16 as one flat [97 x NPOS*192] tensor
(partition row 96 = bias folded into the kh=0 chunk), pulled in per-granule
DMAs on the SP queue only so nothing stalls the weight stream; x and output
DMAs ride the Activation queue.  Each position runs 3 stationary matmuls
([97,64] weights stationary, 16 batch columns moving) accumulating into
PSUM; each granule is cast/copied to fp16 SBUF and stored f-major (host
unscrambles).  The final granules are small and their stores deferred to
minimize the post-stream dependency chain.
"""

import sys

for _p in ("/opt/trn_rl_repo",):
    if _p not in sys.path:
        sys.path.insert(0, _p)

import numpy as np
from contextlib import ExitStack

import concourse.bass as bass
import concourse.bacc as bacc
import concourse.mybir as mybir
import concourse.tile as tile
from concourse.bass_utils import run_bass_kernel_spmd

F32 = mybir.dt.float32
F16 = mybir.dt.float16

B, H, W, C = 16, 64, 64, 32
KH, KW = 3, 3
OH, OW = 62, 62
P_ALL = OH * OW    # 3844
F = 64
NCORES = 8
NPOS = 481         # positions per core (cores 4-7: 480 real + 1 pad)
NVR = 8            # virtual rows (7 full + 47-position partial)
NXR = NVR + 2      # patch tiles staged per core
KP = KW * C + 1    # 97 partitions: 96 contraction rows + bias/ones row
CPP = KH * F       # 192 weight columns per position

# (v, u0, npos) granules; the last ones are small to shrink the tail chain.
GRANULES = []
for _v in range(NVR - 1):
    GRANULES.append((_v, 0, 31))
    GRANULES.append((_v, 31, 31))
GRANULES += [(NVR - 1, 0, 31), (NVR - 1, 31, 12), (NVR - 1, 43, 4)]
assert sum(g[2] for g in GRANULES) == NPOS
N_DEFER = 4        # store DMAs of the last N_DEFER granules issue post-stream


def _p0(c):
    return 481 * c if c < 4 else 1924 + 480 * (c - 4)


_cached = {}


def _build_program():
    if "nc" in _cached:
        return _cached["nc"]

    nc = bacc.Bacc(None)
    # xt[w, kw*32+c, u*16+b] = x[b, w + (p0+u)//62, (p0+u)%62 + kw, c]; row 96 = 1.0
    xt = nc.declare_dram_parameter("xt", [NXR, KP, OW * B], F16, isOutput=False)
    # ks[kw*32+c, j*192 + ch*64 + f] = kernel[p0+j, ch*96+kw*32+c, f];
    # row 96: bias at ch==0, zero at ch 1..2
    ks = nc.declare_dram_parameter("ks", [KP, NPOS * CPP], F16, isOutput=False)
    # out[f, j*16+b]
    out = nc.declare_dram_parameter("out", [F, NPOS * B], F16, isOutput=True)

    with ExitStack() as ctx:
        tc = ctx.enter_context(tile.TileContext(nc))
        tpool = ctx.enter_context(tc.tile_pool(name="tpool", bufs=NXR))
        ktpool = ctx.enter_context(tc.tile_pool(name="ktpool", bufs=4))
        pspool = ctx.enter_context(tc.tile_pool(name="pspool", bufs=4, space="PSUM"))
        stpool = ctx.enter_context(tc.tile_pool(name="stpool", bufs=N_DEFER + 4))

        T = []
        for r in range(NXR):
            t_tile = tpool.tile([KP, OW * B], F16)
            T.append(t_tile)
        # the last tile only feeds the 47-position partial virtual row
        tcols = [OW * B] * (NXR - 1) + [47 * B]
        for r in range(KH):
            nc.scalar.dma_start(T[r][:, : tcols[r]], xt[r, :, : tcols[r]])

        deferred = []
        seen_rows = KH
        for gi, (v, u0, npos) in enumerate(GRANULES):
            j0 = v * OW + u0
            kt = ktpool.tile([KP, 31 * CPP], F16)
            nc.sync.dma_start(
                kt[:, : npos * CPP],
                ks[:, j0 * CPP : (j0 + npos) * CPP],
            )
            if v + KH >= seen_rows and seen_rows < NXR:
                r = seen_rows
                nc.scalar.dma_start(T[r][:, : tcols[r]], xt[r, :, : tcols[r]])
                seen_rows += 1
            ps = pspool.tile([F, 31 * B], F32)
            for owl in range(npos):
                u = u0 + owl
                for ch in range(KH):
                    nc.tensor.matmul(
                        ps[0:F, owl * B : (owl + 1) * B],
                        kt[0:KP, (owl * KH + ch) * F : (owl * KH + ch + 1) * F],
                        T[v + ch][0:KP, u * B : (u + 1) * B],
                        start=(ch == 0),
                        stop=(ch == KH - 1),
                    )
            st = stpool.tile([F, 31 * B], F16)
            nc.vector.tensor_copy(st[:, : npos * B], ps[:, : npos * B])
            dst = out[:, j0 * B : (j0 + npos) * B]
            if gi >= len(GRANULES) - N_DEFER:
                deferred.append((dst, st, npos))
            else:
                nc.scalar.dma_start(dst, st[:, : npos * B])
        for di, (dst, st, npos) in enumerate(deferred):
            # alternate queues so deferred-store issue paths overlap; the
            # very last store rides the (by then idle) SP queue
            eng = nc.sync if di % 2 == 1 else nc.scalar
            eng.dma_start(dst, st[:, : npos * B])

    nc.finalize()
    _cached["nc"] = nc
    return nc


def _shard_inputs(x, kernel, bias):
    x = np.asarray(x, dtype=np.float32)
    kernel = np.asarray(kernel, dtype=np.float32)
    bias = np.asarray(bias, dtype=np.float32)
    kernel16 = kernel.astype(np.float16)   # (P, 288, 64)
    bias16 = bias.astype(np.float16)       # (P, 64)
    x16 = x.astype(np.float16)             # (B, H, W, C)

    xpad = np.zeros((B, H + NXR, W, C), dtype=np.float16)
    xpad[:, :H] = x16

    in_maps = []
    for c in range(NCORES):
        p0 = _p0(c)
        u_arr = np.arange(OW)
        rbase = (p0 + u_arr) // OW   # (62,)
        ucol = (p0 + u_arr) % OW     # (62,)

        xt_c = np.empty((NXR, KP, OW * B), dtype=np.float16)
        xt_c[:, KP - 1, :] = np.float16(1.0)
        for w in range(NXR):
            rows = w + rbase         # (62,)
            for kw in range(KW):
                vals = xpad[:, rows, ucol + kw, :]        # (B, 62, C)
                xt_c[w, kw * C : (kw + 1) * C, :] = vals.transpose(2, 1, 0).reshape(
                    C, OW * B
                )

        nreal = min(NPOS, P_ALL - p0)
        kblk = np.zeros((NPOS, KH, KW * C, F), dtype=np.float16)
        kblk[:nreal] = kernel16[p0 : p0 + nreal].reshape(nreal, KH, KW * C, F)
        ks_c = np.zeros((KP, NPOS * CPP), dtype=np.float16)
        ks_c[: KW * C] = kblk.transpose(2, 0, 1, 3).reshape(KW * C, NPOS * CPP)
        brow = np.zeros((NPOS, KH, F), dtype=np.float16)
        brow[:nreal, 0, :] = bias16[p0 : p0 + nreal]
        ks_c[KP - 1] = brow.reshape(NPOS * CPP)

        in_maps.append({"xt": xt_c, "ks": ks_c})
    return in_maps


def _run(x, kernel, bias, trace=False):
    nc = _build_program()
    in_maps = _shard_inputs(x, kernel, bias)
    res = run_bass_kernel_spmd(nc, in_maps, core_ids=list(range(NCORES)), trace=trace)
    out_flat = np.empty((P_ALL, B, F), dtype=np.float32)
    for c in range(NCORES):
        p0 = _p0(c)
        nreal = min(NPOS, P_ALL - p0)
        o = np.asarray(res.results[c]["out"], dtype=np.float32)  # (64, NPOS*16)
        o = o.reshape(F, NPOS, B).transpose(1, 2, 0)             # (j, b, f)
        out_flat[p0 : p0 + nreal] = o[:nreal]
    out_full = out_flat.reshape(OH, OW, B, F).transpose(2, 0, 1, 3)
    return np.ascontiguousarray(out_full), res


def kernel(x, kernel, bias):
    out, _ = _run(x, kernel, bias, trace=False)
    return out


# revision 19
# speedup vs baseline: 1.0642x; 1.0007x over previous
"""LocallyConnected2D (B=16, H=W=64, C=32, 3x3 valid, F=64) on 8 trn2 cores.

out[b, oh, ow, f] = sum_{kh,kw,c} x[b, oh+kh, ow+kw, c] * kernel[p, (kh,kw,c), f] + bias[p, f]
with p = oh*62+ow, P = 3844.

P is sharded evenly: NPOS=481 consecutive positions per core (cores 4-7 pad
one).  The SPMD program is phase-independent via *virtual rows*: core-local
position j has virtual row v=j//62, column u=j%62, and because
oh(p-62)+1 == oh(p), one patch tile T[w] per virtual row serves as chunk ch
for v=w-ch regardless of the core's row phase — the phase lives entirely in
the host-side layout of T.

Per core: weights stream from HBM in fp16 as one flat [97 x NPOS*192] tensor
(partition row 96 = bias folded into the kh=0 chunk), pulled in per-granule
DMAs on the SP queue only so nothing stalls the weight stream; x and output
DMAs ride the Activation queue.  Each position runs 3 stationary matmuls
([97,64] weights stationary, 16 batch columns moving) accumulating into
PSUM; each granule is cast/copied to fp16 SBUF and stored f-major (host
unscrambles).  The final granules are small and their stores deferred to
minimize the post-stream dependency chain.
"""

import sys

for _p in ("/opt/trn_rl_repo",):
    if _p not in sys.path:
        sys.path.insert(0, _p)

import numpy as np
from contextlib import ExitStack

import concourse.bass as bass
import concourse.bacc as bacc
import concourse.mybir as mybir
import concourse.tile as tile
from concourse.bass_utils import run_bass_kernel_spmd

F32 = mybir.dt.float32
F16 = mybir.dt.float16

B, H, W, C = 16, 64, 64, 32
KH, KW = 3, 3
OH, OW = 62, 62
P_ALL = OH * OW    # 3844
F = 64
NCORES = 8
NPOS = 481         # positions per core (cores 4-7: 480 real + 1 pad)
NVR = 8            # virtual rows (7 full + 47-position partial)
NXR = NVR + 2      # patch tiles staged per core
KP = KW * C + 1    # 97 partitions: 96 contraction rows + bias/ones row
CPP = KH * F       # 192 weight columns per position

# (v, u0, npos) granules; the last ones are small to shrink the tail chain.
GRANULES = []
for _v in range(NVR - 1):
    GRANULES.append((_v, 0, 31))
    GRANULES.append((_v, 31, 31))
GRANULES += [(NVR - 1, 0, 31), (NVR - 1, 31, 12), (NVR - 1, 43, 4)]
assert sum(g[2] for g in GRANULES) == NPOS
N_DEFER = 4        # store DMAs of the last N_DEFER granules issue post-stream


def _p0(c):
    return 481 * c if c < 4 else 1924 + 480 * (c - 4)


_cached = {}


def _build_program():
    if "nc" in _cached:
        return _cached["nc"]

    nc = bacc.Bacc(None)
    # xt[w, kw*32+c, u*16+b] = x[b, w + (p0+u)//62, (p0+u)%62 + kw, c]; row 96 = 1.0
    xt = nc.declare_dram_parameter("xt", [NXR, KP, OW * B], F16, isOutput=False)
    # ks[kw*32+c, j*192 + ch*64 + f] = kernel[p0+j, ch*96+kw*32+c, f];
    # row 96: bias at ch==0, zero at ch 1..2
    ks = nc.declare_dram_parameter("ks", [KP, NPOS * CPP], F16, isOutput=False)
    # out[f, j*16+b]
    out = nc.declare_dram_parameter("out", [F, NPOS * B], F16, isOutput=True)

    with ExitStack() as ctx:
        tc = ctx.enter_context(tile.TileContext(nc))
        tpool = ctx.enter_context(tc.tile_pool(name="tpool", bufs=NXR))
        ktpool = ctx.enter_context(tc.tile_pool(name="ktpool", bufs=4))
        pspool = ctx.enter_context(tc.tile_pool(name="pspool", bufs=4, space="PSUM"))
        stpool = ctx.enter_context(tc.tile_pool(name="stpool", bufs=N_DEFER + 4))

        T = []
        for r in range(NXR):
            t_tile = tpool.tile([KP, OW * B], F16)
            T.append(t_tile)
        # the last tile only feeds the 47-position partial virtual row
        tcols = [OW * B] * (NXR - 1) + [47 * B]
        for r in range(KH):
            nc.scalar.dma_start(T[r][:, : tcols[r]], xt[r, :, : tcols[r]])

        deferred = []
        seen_rows = KH
        for gi, (v, u0, npos) in enumerate(GRANULES):
            j0 = v * OW + u0
            kt = ktpool.tile([KP, 31 * CPP], F16)
            nc.sync.dma_start(
                kt[:, : npos * CPP],
                ks[:, j0 * CPP : (j0 + npos) * CPP],
            )
            if v + KH >= seen_rows and seen_rows < NXR:
                r = seen_rows
                nc.scalar.dma_start(T[r][:, : tcols[r]], xt[r, :, : tcols[r]])
                seen_rows += 1
            ps = pspool.tile([F, 31 * B], F32)
            for owl in range(npos):
                u = u0 + owl
                for ch in range(KH):
                    nc.tensor.matmul(
                        ps[0:F, owl * B : (owl + 1) * B],
                        kt[0:KP, (owl * KH + ch) * F : (owl * KH + ch + 1) * F],
                        T[v + ch][0:KP, u * B : (u + 1) * B],
                        start=(ch == 0),
                        stop=(ch == KH - 1),
                    )
            st = stpool.tile([F, 31 * B], F16)
            nc.vector.tensor_copy(st[:, : npos * B], ps[:, : npos * B])
            dst = out[:, j0 * B : (j0 + npos) * B]
            if gi >= len(GRANULES) - N_DEFER:
                deferred.append((dst, st, npos))
            else:
                nc.scalar.dma_start(dst, st[:, : npos * B])
        for dst, st, npos in deferred:
            # SP keeps these strictly after the last weight load's desc-gen,
            # so they can never stall the weight stream
            nc.sync.dma_start(dst, st[:, : npos * B])

    nc.finalize()
    _cached["nc"] = nc
    return nc


def _shard_inputs(x, kernel, bias):
    x = np.asarray(x, dtype=np.float32)
    kernel = np.asarray(kernel, dtype=np.float32)
    bias = np.asarray(bias, dtype=np.float32)
    kernel16 = kernel.astype(np.float16)   # (P, 288, 64)
    bias16 = bias.astype(np.float16)       # (P, 64)
    x16 = x.astype(np.float16)             # (B, H, W, C)

    xpad = np.zeros((B, H + NXR, W, C), dtype=np.float16)
    xpad[:, :H] = x16

    in_maps = []
    for c in range(NCORES):
        p0 = _p0(c)
        u_arr = np.arange(OW)
        rbase = (p0 + u_arr) // OW   # (62,)
        ucol = (p0 + u_arr) % OW     # (62,)

        xt_c = np.empty((NXR, KP, OW * B), dtype=np.float16)
        xt_c[:, KP - 1, :] = np.float16(1.0)
        for w in range(NXR):
            rows = w + rbase         # (62,)
            for kw in range(KW):
                vals = xpad[:, rows, ucol + kw, :]        # (B, 62, C)
                xt_c[w, kw * C : (kw + 1) * C, :] = vals.transpose(2, 1, 0).reshape(
                    C, OW * B
                )

        nreal = min(NPOS, P_ALL - p0)
        kblk = np.zeros((NPOS, KH, KW * C, F), dtype=np.float16)
        kblk[:nreal] = kernel16[p0 : p0 + nreal].reshape(nreal, KH, KW * C, F)
        ks_c = np.zeros((KP, NPOS * CPP), dtype=np.float16)
        ks_c[: KW * C] = kblk.transpose(2, 0, 1, 3).reshape(KW * C, NPOS * CPP)
        brow = np.zeros((NPOS, KH, F), dtype=np.float16)
        brow[:nreal, 0, :] = bias16[p0 : p0 + nreal]
        ks_c[KP - 1] = brow.reshape(NPOS * CPP)

        in_maps.append({"xt": xt_c, "ks": ks_c})
    return in_maps


def _run(x, kernel, bias, trace=False):
    nc = _build_program()
    in_maps = _shard_inputs(x, kernel, bias)
    res = run_bass_kernel_spmd(nc, in_maps, core_ids=list(range(NCORES)), trace=trace)
    out_flat = np.empty((P_ALL, B, F), dtype=np.float32)
    for c in range(NCORES):
        p0 = _p0(c)
        nreal = min(NPOS, P_ALL - p0)
        o = np.asarray(res.results[c]["out"], dtype=np.float32)  # (64, NPOS*16)
        o = o.reshape(F, NPOS, B).transpose(1, 2, 0)             # (j, b, f)
        out_flat[p0 : p0 + nreal] = o[:nreal]
    out_full = out_flat.reshape(OH, OW, B, F).transpose(2, 0, 1, 3)
    return np.ascontiguousarray(out_full), res


def kernel(x, kernel, bias):
    out, _ = _run(x, kernel, bias, trace=False)
    return out


# revision 20
# speedup vs baseline: 1.0698x; 1.0052x over previous
"""LocallyConnected2D (B=16, H=W=64, C=32, 3x3 valid, F=64) on 8 trn2 cores.

out[b, oh, ow, f] = sum_{kh,kw,c} x[b, oh+kh, ow+kw, c] * kernel[p, (kh,kw,c), f] + bias[p, f]
with p = oh*62+ow, P = 3844.

P is sharded evenly: NPOS=481 consecutive positions per core (cores 4-7 pad
one).  The SPMD program is phase-independent via *virtual rows*: core-local
position j has virtual row v=j//62, column u=j%62, and because
oh(p-62)+1 == oh(p), one patch tile T[w] per virtual row serves as chunk ch
for v=w-ch regardless of the core's row phase — the phase lives entirely in
the host-side layout of T.

Per core: weights stream from HBM in fp16 as one flat [97 x NPOS*192] tensor
(partition row 96 = bias folded into the kh=0 chunk), pulled in per-granule
DMAs on the SP queue only so nothing stalls the weight stream; x and output
DMAs ride the Activation queue.  Each position runs 3 stationary matmuls
([97,64] weights stationary, 16 batch columns moving) accumulating into
PSUM; each granule is cast/copied to fp16 SBUF and stored f-major (host
unscrambles).  The final granules are small and their stores deferred to
minimize the post-stream dependency chain.
"""

import sys

for _p in ("/opt/trn_rl_repo",):
    if _p not in sys.path:
        sys.path.insert(0, _p)

import numpy as np
from contextlib import ExitStack

import concourse.bass as bass
import concourse.bacc as bacc
import concourse.mybir as mybir
import concourse.tile as tile
from concourse.bass_utils import run_bass_kernel_spmd

F32 = mybir.dt.float32
F16 = mybir.dt.float16

B, H, W, C = 16, 64, 64, 32
KH, KW = 3, 3
OH, OW = 62, 62
P_ALL = OH * OW    # 3844
F = 64
NCORES = 8
NPOS = 481         # positions per core (cores 4-7: 480 real + 1 pad)
NVR = 8            # virtual rows (7 full + 47-position partial)
NXR = NVR + 2      # patch tiles staged per core
KP = KW * C + 1    # 97 partitions: 96 contraction rows + bias/ones row
CPP = KH * F       # 192 weight columns per position

# (v, u0, npos) granules; the last ones are small to shrink the tail chain.
GRANULES = []
for _v in range(NVR - 1):
    GRANULES.append((_v, 0, 31))
    GRANULES.append((_v, 31, 31))
GRANULES += [(NVR - 1, 0, 31), (NVR - 1, 31, 12), (NVR - 1, 43, 4)]
assert sum(g[2] for g in GRANULES) == NPOS
N_DEFER = 4        # store DMAs of the last N_DEFER granules issue post-stream


def _p0(c):
    return 481 * c if c < 4 else 1924 + 480 * (c - 4)


_cached = {}


def _build_program():
    if "nc" in _cached:
        return _cached["nc"]

    nc = bacc.Bacc(None)
    # xt[w, kw*32+c, u*16+b] = x[b, w + (p0+u)//62, (p0+u)%62 + kw, c]; row 96 = 1.0
    xt = nc.declare_dram_parameter("xt", [NXR, KP, OW * B], F16, isOutput=False)
    # ks[kw*32+c, j*192 + ch*64 + f] = kernel[p0+j, ch*96+kw*32+c, f];
    # row 96: bias at ch==0, zero at ch 1..2
    ks = nc.declare_dram_parameter("ks", [KW * C, NPOS * CPP], F16, isOutput=False)
    # bs[k, g*64+f] = bias[p0 + j0(g) + k, f]; ind[k, (owl,b)] = (k == owl)
    bs = nc.declare_dram_parameter("bs", [31, len(GRANULES) * F], F16, isOutput=False)
    ind = nc.declare_dram_parameter("ind", [31, 31 * B], F16, isOutput=False)
    # out[f, j*16+b]
    out = nc.declare_dram_parameter("out", [F, NPOS * B], F16, isOutput=True)

    with ExitStack() as ctx:
        tc = ctx.enter_context(tile.TileContext(nc))
        tpool = ctx.enter_context(tc.tile_pool(name="tpool", bufs=NXR))
        ktpool = ctx.enter_context(tc.tile_pool(name="ktpool", bufs=4))
        pspool = ctx.enter_context(tc.tile_pool(name="pspool", bufs=4, space="PSUM"))
        stpool = ctx.enter_context(tc.tile_pool(name="stpool", bufs=N_DEFER + 4))

        cpool = ctx.enter_context(tc.tile_pool(name="cpool", bufs=1))
        bs_t = cpool.tile([31, len(GRANULES) * F], F16)
        ind_t = cpool.tile([31, 31 * B], F16)
        nc.scalar.dma_start(bs_t[:, :], bs[:, :])
        nc.scalar.dma_start(ind_t[:, :], ind[:, :])

        T = []
        for r in range(NXR):
            t_tile = tpool.tile([KP, OW * B], F16)
            T.append(t_tile)
        # the last tile only feeds the 47-position partial virtual row
        tcols = [OW * B] * (NXR - 1) + [47 * B]
        for r in range(KH):
            nc.scalar.dma_start(T[r][:, : tcols[r]], xt[r, :, : tcols[r]])

        deferred = []
        seen_rows = KH
        for gi, (v, u0, npos) in enumerate(GRANULES):
            j0 = v * OW + u0
            kt = ktpool.tile([KW * C, 31 * CPP], F16)
            nc.sync.dma_start(
                kt[:, : npos * CPP],
                ks[:, j0 * CPP : (j0 + npos) * CPP],
            )
            if v + KH >= seen_rows and seen_rows < NXR:
                r = seen_rows
                nc.scalar.dma_start(T[r][:, : tcols[r]], xt[r, :, : tcols[r]])
                seen_rows += 1
            ps = pspool.tile([F, 31 * B], F32)
            nc.tensor.matmul(
                ps[0:F, 0 : npos * B],
                bs_t[0:npos, gi * F : (gi + 1) * F],
                ind_t[0:npos, 0 : npos * B],
                start=True,
                stop=False,
                skip_group_check=True,
            )
            for owl in range(npos):
                u = u0 + owl
                for ch in range(KH):
                    nc.tensor.matmul(
                        ps[0:F, owl * B : (owl + 1) * B],
                        kt[0 : KW * C, (owl * KH + ch) * F : (owl * KH + ch + 1) * F],
                        T[v + ch][0 : KW * C, u * B : (u + 1) * B],
                        start=False,
                        stop=(ch == KH - 1),
                        skip_group_check=True,
                    )
            st = stpool.tile([F, 31 * B], F16)
            nc.vector.tensor_copy(st[:, : npos * B], ps[:, : npos * B])
            dst = out[:, j0 * B : (j0 + npos) * B]
            if gi >= len(GRANULES) - N_DEFER:
                deferred.append((dst, st, npos))
            else:
                nc.scalar.dma_start(dst, st[:, : npos * B])
        for dst, st, npos in deferred:
            # SP keeps these strictly after the last weight load's desc-gen,
            # so they can never stall the weight stream
            nc.sync.dma_start(dst, st[:, : npos * B])

    nc.finalize()
    _cached["nc"] = nc
    return nc


def _shard_inputs(x, kernel, bias):
    x = np.asarray(x, dtype=np.float32)
    kernel = np.asarray(kernel, dtype=np.float32)
    bias = np.asarray(bias, dtype=np.float32)
    kernel16 = kernel.astype(np.float16)   # (P, 288, 64)
    bias16 = bias.astype(np.float16)       # (P, 64)
    x16 = x.astype(np.float16)             # (B, H, W, C)

    xpad = np.zeros((B, H + NXR, W, C), dtype=np.float16)
    xpad[:, :H] = x16

    in_maps = []
    for c in range(NCORES):
        p0 = _p0(c)
        u_arr = np.arange(OW)
        rbase = (p0 + u_arr) // OW   # (62,)
        ucol = (p0 + u_arr) % OW     # (62,)

        xt_c = np.empty((NXR, KP, OW * B), dtype=np.float16)
        xt_c[:, KP - 1, :] = np.float16(1.0)
        for w in range(NXR):
            rows = w + rbase         # (62,)
            for kw in range(KW):
                vals = xpad[:, rows, ucol + kw, :]        # (B, 62, C)
                xt_c[w, kw * C : (kw + 1) * C, :] = vals.transpose(2, 1, 0).reshape(
                    C, OW * B
                )

        nreal = min(NPOS, P_ALL - p0)
        kblk = np.zeros((NPOS, KH, KW * C, F), dtype=np.float16)
        kblk[:nreal] = kernel16[p0 : p0 + nreal].reshape(nreal, KH, KW * C, F)
        ks_c = kblk.transpose(2, 0, 1, 3).reshape(KW * C, NPOS * CPP).copy()
        bpad = np.zeros((NPOS, F), dtype=np.float16)
        bpad[:nreal] = bias16[p0 : p0 + nreal]
        bs_c = np.zeros((31, len(GRANULES) * F), dtype=np.float16)
        for g, (_v, _u0, _np) in enumerate(GRANULES):
            _j0 = _v * OW + _u0
            bs_c[:_np, g * F : (g + 1) * F] = bpad[_j0 : _j0 + _np]
        ind_c = np.zeros((31, 31 * B), dtype=np.float16)
        for k in range(31):
            ind_c[k, k * B : (k + 1) * B] = np.float16(1.0)

        in_maps.append({"xt": xt_c, "ks": ks_c, "bs": bs_c, "ind": ind_c})
    return in_maps


def _run(x, kernel, bias, trace=False):
    nc = _build_program()
    in_maps = _shard_inputs(x, kernel, bias)
    res = run_bass_kernel_spmd(nc, in_maps, core_ids=list(range(NCORES)), trace=trace)
    out_flat = np.empty((P_ALL, B, F), dtype=np.float32)
    for c in range(NCORES):
        p0 = _p0(c)
        nreal = min(NPOS, P_ALL - p0)
        o = np.asarray(res.results[c]["out"], dtype=np.float32)  # (64, NPOS*16)
        o = o.reshape(F, NPOS, B).transpose(1, 2, 0)             # (j, b, f)
        out_flat[p0 : p0 + nreal] = o[:nreal]
    out_full = out_flat.reshape(OH, OW, B, F).transpose(2, 0, 1, 3)
    return np.ascontiguousarray(out_full), res


def kernel(x, kernel, bias):
    out, _ = _run(x, kernel, bias, trace=False)
    return out
